# revision 4
# baseline (speedup 1.0000x reference)
"""AFT-local attention on 8 Trainium2 NeuronCores.

Reference (per batch element b, S=2048, D=512, window=128):
    query = q @ Wq.T + bq;  Q_ = sigmoid(query)
    key_p = k @ Wk.T + bk;  ek = exp(key_p)
    value = v @ Wv.T + bv;  ekv = ek * value
    ew    = exp(w_bias * local_mask)          # S x S, == 1 outside the band
    num_raw = ew @ ekv ; den = ew @ ek        # dense S x S einsums
    num  = Q_ * num_raw;  x = num / den
    out1 = x @ out_w.T + out_b
    return (out1, num)

Key decomposition: ew = 1 + (exp(wb_masked) - 1) restricted to the band
|i-j| < 128, so  ew @ Y = colsum(Y) + EWM1_band @ Y  where the banded part
only touches 3 column tiles of 128 per row tile of 128 (46 block matmuls
instead of 256 dense ones).

Sharding: pure data-parallel; batch B=8 -> one batch element per core.

Matmuls run as float32r (single-pass fp32 on the PE systolic array,
~1.5e-4 rel err measured on silicon vs fp32 numpy).
"""

import sys

if "/opt/trn_rl_repo" not in sys.path:
    sys.path.insert(0, "/opt/trn_rl_repo")

import numpy as np

import concourse.bacc as bacc
import concourse.mybir as mybir
import concourse.tile as tile
from concourse.bass import ts
from concourse.bass_utils import run_bass_kernel_spmd
from concourse.masks import make_identity

F32 = mybir.dt.float32
F32R = mybir.dt.float32r
EXP = mybir.ActivationFunctionType.Exp
SIGMOID = mybir.ActivationFunctionType.Sigmoid

S = 2048
D = 512
P = 128
NT = S // P  # 16 sequence tiles
NC = D // P  # 4 contraction chunks of the model dim
N_CORES = 8


def _band_blocks(i):
    """Valid (jl, j) column-tile neighbors for row tile i."""
    return [(jl, i - 1 + jl) for jl in range(3) if 0 <= i - 1 + jl < NT]


def build(with_biases):
    nc = bacc.Bacc(None, target_bir_lowering=False, debug=False)

    qT_d = nc.dram_tensor("qT", [D, S], F32, kind="ExternalInput")
    kT_d = nc.dram_tensor("kT", [D, S], F32, kind="ExternalInput")
    vT_d = nc.dram_tensor("vT", [D, S], F32, kind="ExternalInput")
    wqT_d = nc.dram_tensor("wqT", [D, D], F32, kind="ExternalInput")
    wkT_d = nc.dram_tensor("wkT", [D, D], F32, kind="ExternalInput")
    wvT_d = nc.dram_tensor("wvT", [D, D], F32, kind="ExternalInput")
    woT_d = nc.dram_tensor("woT", [D, D], F32, kind="ExternalInput")
    band_d = nc.dram_tensor("band", [NT, P, 3 * P], F32, kind="ExternalInput")
    if with_biases:
        # rows: bq, bk, bv, bo
        bias_d = nc.dram_tensor("biases", [4, D], F32, kind="ExternalInput")
    out1_d = nc.dram_tensor("out1", [S, D], F32, kind="ExternalOutput")
    num_d = nc.dram_tensor("num", [S, D], F32, kind="ExternalOutput")

    with tile.TileContext(nc) as tc:
        with (
            tc.tile_pool(name="consts", bufs=1) as consts,
            tc.tile_pool(name="weights", bufs=1) as wpool,
            tc.tile_pool(name="ekk_pool", bufs=1) as ekkpool,
            tc.tile_pool(name="kv_in", bufs=3) as kvpool,
            tc.tile_pool(name="q_in", bufs=3) as qpool,
            tc.tile_pool(name="work", bufs=3) as work,
            tc.tile_pool(name="outs", bufs=3) as outs,
            tc.tile_pool(name="psum", bufs=1, space="PSUM") as psum,
        ):
            # ---- constants ----
            identity = consts.tile([P, P], F32)
            make_identity(nc, identity)
            ones_f32 = consts.tile([P, P], F32)
            nc.any.memset(ones_f32, 1.0)
            ones_col = consts.tile([P, 1], F32R)  # lhsT for column sums
            nc.vector.tensor_copy(ones_col, ones_f32[:, 0:1])
            ones_row = consts.tile([1, P], F32R)  # lhsT for partition bcast
            nc.vector.tensor_copy(ones_row, ones_f32[0:1, :])

            # ---- weights: [din(4x128), dout 512] as [128, 4, 512] ----
            w_sb = {}
            for name, d in (
                ("wq", wqT_d),
                ("wk", wkT_d),
                ("wv", wvT_d),
                ("wo", woT_d),
            ):
                t = wpool.tile([P, NC, D], F32R, tag=f"w_{name}")
                nc.sync.dma_start(
                    t, d[:, :].bitcast(F32R).rearrange("(c p) n -> p c n", p=P)
                )
                w_sb[name] = t

            if with_biases:
                bias_sb = consts.tile([4, D], F32R)
                nc.sync.dma_start(bias_sb, bias_d[:, :].bitcast(F32R))

            # ---- band: [16, 128 t', 384] -> sbuf [128 t', 16, 384] ----
            band_sb = wpool.tile([P, NT, 3 * P], F32, tag="band")
            nc.sync.dma_start(
                band_sb, band_d[:, :, :].rearrange("i p f -> p i f")
            )

            # ---- ekk: per seq-tile j, [ekv | ek] along free dim ----
            ekk = ekkpool.tile([P, NT, 2 * D], F32R)

            def proj_psum(xT_tile, wname, bias_row):
                """psum [128, 512] = (x @ W.T + b) for one 128-seq tile."""
                ps = psum.tile([P, D], F32, tag="ps", bufs=3, name="proj_ps")
                for c in range(NC):
                    nc.tensor.matmul(
                        ps,
                        xT_tile[:, c, :],
                        w_sb[wname][:, c, :],
                        start=(c == 0),
                        stop=(c == NC - 1 and bias_row is None),
                    )
                if bias_row is not None:
                    nc.tensor.matmul(
                        ps,
                        ones_row[:, :],
                        bias_sb[bias_row : bias_row + 1, :],
                        start=False,
                        stop=True,
                    )
                return ps

            # ---- phase B: ek / ekv + column sums ----
            csum_ps = {}
            if True:
                for h in range(2):
                    csum_ps[h] = psum.tile([1, D], F32, tag="cs", bufs=2, name=f"cs{h}")
                for j in range(NT):
                    kT_t = kvpool.tile([P, NC, P], F32R, tag="kT_t")
                    nc.sync.dma_start(
                        kT_t,
                        kT_d[:, ts(j, P)]
                        .bitcast(F32R)
                        .rearrange("(c p) t -> p c t", p=P),
                    )
                    vT_t = kvpool.tile([P, NC, P], F32R, tag="vT_t")
                    nc.sync.dma_start(
                        vT_t,
                        vT_d[:, ts(j, P)]
                        .bitcast(F32R)
                        .rearrange("(c p) t -> p c t", p=P),
                    )

                    keyp_ps = proj_psum(kT_t, "wk", 1 if with_biases else None)
                    ek_view = ekk[:, j, D : 2 * D]
                    nc.scalar.activation(ek_view, keyp_ps, EXP)

                    val_ps = proj_psum(vT_t, "wv", 2 if with_biases else None)
                    nc.vector.tensor_mul(
                        ekk[:, j, 0:D], ek_view.bitcast(F32), val_ps
                    )

                    # column sums across the whole sequence (shared by all
                    # output row tiles): ones^T @ ekk
                    for h in range(2):
                        nc.tensor.matmul(
                            csum_ps[h],
                            ones_col[:, :],
                            ekk[:, j, h * D : (h + 1) * D],
                            start=(j == 0),
                            stop=(j == NT - 1),
                        )

                csum_sb = consts.tile([1, 2 * D], F32R)
                for h in range(2):
                    nc.scalar.copy(csum_sb[:, h * D : (h + 1) * D], csum_ps[h])

            # ---- phase C: per output row tile ----
            for i in range(NT):
                qT_t = qpool.tile([P, NC, P], F32R, tag="qT_t")
                nc.sync.dma_start(
                    qT_t,
                    qT_d[:, ts(i, P)]
                    .bitcast(F32R)
                    .rearrange("(c p) t -> p c t", p=P),
                )
                query_ps = proj_psum(qT_t, "wq", 0 if with_biases else None)
                q_sb = work.tile([P, D], F32, tag="q_sb")
                nc.scalar.activation(q_sb, query_ps, SIGMOID)

                # ewm1 = exp(masked w_bias) - 1 for the 3 neighbor blocks
                ew_sb = work.tile([P, 3 * P], F32, tag="ew_sb")
                nc.scalar.activation(ew_sb, band_sb[:, i, :], EXP)
                ewm1 = work.tile([P, 3 * P], F32R, tag="ewm1")
                nc.vector.tensor_scalar_add(ewm1, ew_sb, -1.0)

                halves = []
                for h in range(2):  # 0: num (ekv), 1: den (ek)
                    ps = psum.tile([P, D], F32, tag="bps", bufs=2, name=f"bps{h}")
                    blocks = _band_blocks(i)
                    for bi, (jl, j) in enumerate(blocks):
                        nc.tensor.matmul(
                            ps,
                            ewm1[:, ts(jl, P)],
                            ekk[:, j, h * D : (h + 1) * D],
                            start=(bi == 0),
                            stop=False,
                        )
                    # + colsum broadcast to all 128 partitions (K=1 matmul)
                    nc.tensor.matmul(
                        ps,
                        ones_row[:, :],
                        csum_sb[:, h * D : (h + 1) * D],
                        start=False,
                        stop=True,
                    )
                    halves.append(ps)
                num_ps, den_ps = halves

                rcp_sb = work.tile([P, D], F32, tag="rcp")
                nc.vector.reciprocal(rcp_sb, den_ps)
                num_sb = outs.tile([P, D], F32, tag="num_sb")
                nc.vector.tensor_mul(num_sb, q_sb, num_ps)
                nc.sync.dma_start(num_d[ts(i, P), :], num_sb)
                x_sb = work.tile([P, D], F32, tag="x_sb")
                nc.vector.tensor_mul(x_sb, num_sb, rcp_sb)

                # xT via PE transpose, then the output projection
                xT_psum = psum.tile([P, NC, P], F32, tag="xT", bufs=1, name="xT_psum")
                for c in range(NC):
                    nc.tensor.transpose(
                        xT_psum[:, c, :], x_sb[:, ts(c, P)], identity
                    )
                xT_sb = work.tile([P, NC, P], F32R, tag="xT_sb")
                nc.scalar.copy(xT_sb, xT_psum)

                out_ps = proj_psum(xT_sb, "wo", 3 if with_biases else None)
                out_sb = outs.tile([P, D], F32, tag="out_sb")
                nc.scalar.copy(out_sb, out_ps)
                nc.sync.dma_start(out1_d[ts(i, P), :], out_sb)

    nc.finalize()
    return nc


def _pack_band(w_bias, local_mask):
    """[16, 128, 384] transposed masked-bias blocks: pack[i, t', jl*128+s']
    = (w_bias*mask)[i*128+s', (i-1+jl)*128+t']."""
    wbm = (np.asarray(w_bias, np.float32) * np.asarray(local_mask, np.float32))
    pack = np.zeros((NT, P, 3 * P), np.float32)
    for i in range(NT):
        for jl, j in _band_blocks(i):
            blk = wbm[i * P : (i + 1) * P, j * P : (j + 1) * P]
            pack[i, :, jl * P : (jl + 1) * P] = blk.T
    # anything |i-j| >= 2 tiles must be masked out for the decomposition
    for i in range(NT):
        lo = max(0, (i - 1) * P)
        hi = min(S, (i + 2) * P)
        row = wbm[i * P : (i + 1) * P]
        if row[:, :lo].any() or row[:, hi:].any():
            raise ValueError("w_bias*mask has support outside the 3-tile band")
    return pack


_CACHE = {}


def _get_nc(with_biases):
    key = bool(with_biases)
    if key not in _CACHE:
        _CACHE[key] = build(key)
    return _CACHE[key]


def run(inputs, trace=False):
    q = np.asarray(inputs["q"], np.float32)
    k = np.asarray(inputs["k"], np.float32)
    v = np.asarray(inputs["v"], np.float32)
    B = q.shape[0]
    assert B == N_CORES and q.shape[1:] == (S, D)

    biases = np.stack(
        [
            np.asarray(inputs["Wq_b"], np.float32),
            np.asarray(inputs["Wk_b"], np.float32),
            np.asarray(inputs["Wv_b"], np.float32),
            np.asarray(inputs["out_b"], np.float32),
        ]
    )
    with_biases = bool(np.any(biases))

    shared = {
        "wqT": np.ascontiguousarray(np.asarray(inputs["Wq_w"], np.float32).T),
        "wkT": np.ascontiguousarray(np.asarray(inputs["Wk_w"], np.float32).T),
        "wvT": np.ascontiguousarray(np.asarray(inputs["Wv_w"], np.float32).T),
        "woT": np.ascontiguousarray(np.asarray(inputs["out_w"], np.float32).T),
        "band": _pack_band(inputs["w_bias"], inputs["local_mask"]),
    }
    if with_biases:
        shared["biases"] = biases

    in_maps = []
    for b in range(B):
        m = dict(shared)
        m["qT"] = np.ascontiguousarray(q[b].T)
        m["kT"] = np.ascontiguousarray(k[b].T)
        m["vT"] = np.ascontiguousarray(v[b].T)
        in_maps.append(m)

    nc = _get_nc(with_biases)
    res = run_bass_kernel_spmd(
        nc, in_maps, core_ids=list(range(N_CORES)), trace=trace
    )
    out1 = np.stack([res.results[b]["out1"] for b in range(B)])
    num = np.stack([res.results[b]["num"] for b in range(B)])
    return (out1, num), res


def kernel(**inputs):
    (out1, num), _ = run(inputs, trace=False)
    return (out1, num)


# revision 6
# speedup vs baseline: 1.2646x; 1.2646x over previous
"""AFT-local attention on 8 Trainium2 NeuronCores.

Reference (per batch element b, S=2048, D=512, window=128):
    query = q @ Wq.T + bq;  Q_ = sigmoid(query)
    key_p = k @ Wk.T + bk;  ek = exp(key_p)
    value = v @ Wv.T + bv;  ekv = ek * value
    ew    = exp(w_bias * local_mask)          # S x S, == 1 outside the band
    num_raw = ew @ ekv ; den = ew @ ek        # dense S x S einsums
    num  = Q_ * num_raw;  x = num / den
    out1 = x @ out_w.T + out_b
    return (out1, num)

Key decomposition: ew = 1 + (exp(wb_masked) - 1) restricted to the band
|i-j| < 128, so  ew @ Y = colsum(Y) + EWM1_band @ Y  where the banded part
only touches <=3 column tiles of 128 per row tile of 128 (46 block matmuls
instead of 256 dense ones).  colsum(Y) is one ones-vector matmul per
sequence tile, and its broadcast back to all 128 output partitions is a
K=1 matmul accumulated into the same PSUM tile.

Sharding: pure data-parallel; batch B=8 -> one batch element per core.

Matmuls run as float32r (single-pass fp32 on the PE systolic array,
~1.5e-4 rel err measured on silicon).  All DMA sources are host-packed so
every partition line is a single contiguous burst.
"""

import sys

if "/opt/trn_rl_repo" not in sys.path:
    sys.path.insert(0, "/opt/trn_rl_repo")

import numpy as np

import concourse.bacc as bacc
import concourse.mybir as mybir
import concourse.tile as tile
from concourse.bass import ts
from concourse.bass_utils import run_bass_kernel_spmd
from concourse.masks import make_identity

F32 = mybir.dt.float32
F32R = mybir.dt.float32r
EXP = mybir.ActivationFunctionType.Exp
SIGMOID = mybir.ActivationFunctionType.Sigmoid

S = 2048
D = 512
P = 128
NT = S // P  # 16 sequence tiles
NC = D // P  # 4 contraction chunks of the model dim
N_CORES = 8


def _band_blocks(i):
    """Valid (jl, j) column-tile neighbors for row tile i."""
    return [(jl, i - 1 + jl) for jl in range(3) if 0 <= i - 1 + jl < NT]


def build(with_biases):
    nc = bacc.Bacc(None, target_bir_lowering=False, debug=False)

    # host-packed layouts: every [128, ...] DMA partition line is contiguous
    qP_d = nc.dram_tensor("qP", [NT, P, NC * P], F32, kind="ExternalInput")
    kP_d = nc.dram_tensor("kP", [NT, P, NC * P], F32, kind="ExternalInput")
    vP_d = nc.dram_tensor("vP", [NT, P, NC * P], F32, kind="ExternalInput")
    wq_d = nc.dram_tensor("wqP", [P, NC * D], F32, kind="ExternalInput")
    wk_d = nc.dram_tensor("wkP", [P, NC * D], F32, kind="ExternalInput")
    wv_d = nc.dram_tensor("wvP", [P, NC * D], F32, kind="ExternalInput")
    wo_d = nc.dram_tensor("woP", [P, NC * D], F32, kind="ExternalInput")
    band_d = nc.dram_tensor("bandP", [P, NT * 3 * P], F32, kind="ExternalInput")
    if with_biases:
        # rows: bq, bk, bv, bo
        bias_d = nc.dram_tensor("biases", [4, D], F32, kind="ExternalInput")
    out1_d = nc.dram_tensor("out1", [S, D], F32, kind="ExternalOutput")
    num_d = nc.dram_tensor("num", [S, D], F32, kind="ExternalOutput")

    with tile.TileContext(nc) as tc:
        with (
            tc.tile_pool(name="consts", bufs=1) as consts,
            tc.tile_pool(name="weights", bufs=1) as wpool,
            tc.tile_pool(name="ekk_pool", bufs=1) as ekkpool,
            tc.tile_pool(name="kv_in", bufs=3) as kvpool,
            tc.tile_pool(name="q_in", bufs=3) as qpool,
            tc.tile_pool(name="work", bufs=3) as work,
            tc.tile_pool(name="outs", bufs=3) as outs,
            tc.tile_pool(name="psum", bufs=1, space="PSUM") as psum,
        ):
            # ---- weights first: phase B can start as soon as these land ----
            w_sb = {}
            for name, d in (("wk", wk_d), ("wv", wv_d), ("wq", wq_d), ("wo", wo_d)):
                t = wpool.tile([P, NC, D], F32R, tag=f"w_{name}", name=f"w_{name}")
                nc.sync.dma_start(
                    t, d[:, :].bitcast(F32R).rearrange("p (c n) -> p c n", c=NC)
                )
                w_sb[name] = t

            # ---- constants ----
            identity = consts.tile([P, P], F32)
            make_identity(nc, identity)
            ones_f32 = consts.tile([P, 1], F32)
            nc.gpsimd.memset(ones_f32, 1.0)
            ones_col = consts.tile([P, 1], F32R)  # lhsT for column sums
            nc.vector.tensor_copy(ones_col, ones_f32)
            ones_row = consts.tile([1, P], F32R)  # lhsT for partition bcast
            nc.vector.tensor_copy(ones_row, ones_f32[0:1, 0:1].broadcast_to([1, P]))

            if with_biases:
                bias_sb = consts.tile([4, D], F32R)
                nc.sync.dma_start(bias_sb, bias_d[:, :].bitcast(F32R))

            # ---- band -> ewm1, computed in place in two whole-tensor ops ----
            ewm1 = wpool.tile([P, NT, 3 * P], F32R, tag="ewm1", name="ewm1")
            nc.gpsimd.dma_start(
                ewm1, band_d[:, :].bitcast(F32R).rearrange("p (i f) -> p i f", i=NT)
            )
            ew_flat = ewm1.rearrange("p i f -> p (i f)")
            nc.scalar.activation(ew_flat, ew_flat.bitcast(F32), EXP)
            nc.vector.tensor_scalar_add(ew_flat, ew_flat.bitcast(F32), -1.0)

            # ---- ekk: per seq-tile j, [ekv | ek] along free dim ----
            ekk = ekkpool.tile([P, NT, 2 * D], F32R)

            def proj_psum(xT_tile, wname, bias_row):
                """psum [128, 512] = (x @ W.T + b) for one 128-seq tile."""
                ps = psum.tile([P, D], F32, tag="ps", bufs=3, name="proj_ps")
                for c in range(NC):
                    nc.tensor.matmul(
                        ps,
                        xT_tile[:, c, :],
                        w_sb[wname][:, c, :],
                        start=(c == 0),
                        stop=(c == NC - 1 and bias_row is None),
                    )
                if bias_row is not None:
                    nc.tensor.matmul(
                        ps,
                        ones_row[:, :],
                        bias_sb[bias_row : bias_row + 1, :],
                        start=False,
                        stop=True,
                    )
                return ps

            # ---- phase B: ek / ekv + column sums ----
            csum_ps = {}
            for h in range(2):
                csum_ps[h] = psum.tile([1, D], F32, tag="xT", bufs=2, name=f"cs{h}")
            for j in range(NT):
                kT_t = kvpool.tile([P, NC, P], F32R, tag="kT_t")
                nc.sync.dma_start(
                    kT_t, kP_d[j].bitcast(F32R).rearrange("p (c t) -> p c t", c=NC)
                )
                vT_t = kvpool.tile([P, NC, P], F32R, tag="vT_t")
                nc.sync.dma_start(
                    vT_t, vP_d[j].bitcast(F32R).rearrange("p (c t) -> p c t", c=NC)
                )

                keyp_ps = proj_psum(kT_t, "wk", 1 if with_biases else None)
                ek_view = ekk[:, j, D : 2 * D]
                nc.scalar.activation(ek_view, keyp_ps, EXP)

                val_ps = proj_psum(vT_t, "wv", 2 if with_biases else None)
                nc.vector.tensor_mul(ekk[:, j, 0:D], ek_view.bitcast(F32), val_ps)

                # column sums across the whole sequence (shared by all
                # output row tiles): ones^T @ ekk
                for h in range(2):
                    nc.tensor.matmul(
                        csum_ps[h],
                        ones_col[:, :],
                        ekk[:, j, h * D : (h + 1) * D],
                        start=(j == 0),
                        stop=(j == NT - 1),
                    )

            csum_sb = consts.tile([1, 2 * D], F32R)
            for h in range(2):
                nc.vector.tensor_copy(csum_sb[:, h * D : (h + 1) * D], csum_ps[h])

            # ---- phase C: per output row tile ----
            for i in range(NT):
                qT_t = qpool.tile([P, NC, P], F32R, tag="qT_t")
                nc.sync.dma_start(
                    qT_t, qP_d[i].bitcast(F32R).rearrange("p (c t) -> p c t", c=NC)
                )
                query_ps = proj_psum(qT_t, "wq", 0 if with_biases else None)
                q_sb = work.tile([P, D], F32, tag="q_sb")
                nc.scalar.activation(q_sb, query_ps, SIGMOID)

                halves = []
                for h in range(2):  # 0: num (ekv), 1: den (ek)
                    ps = psum.tile([P, D], F32, tag="bps", bufs=3, name=f"bps{h}")
                    blocks = _band_blocks(i)
                    for bi, (jl, j) in enumerate(blocks):
                        nc.tensor.matmul(
                            ps,
                            ewm1[:, i, ts(jl, P)],
                            ekk[:, j, h * D : (h + 1) * D],
                            start=(bi == 0),
                            stop=False,
                        )
                    # + colsum broadcast to all 128 partitions (K=1 matmul)
                    nc.tensor.matmul(
                        ps,
                        ones_row[:, :],
                        csum_sb[:, h * D : (h + 1) * D],
                        start=False,
                        stop=True,
                    )
                    halves.append(ps)
                num_ps, den_ps = halves

                rcp_sb = work.tile([P, D], F32, tag="rcp")
                nc.vector.reciprocal_approx_fast(out=rcp_sb, in_=den_ps)
                num_sb = outs.tile([P, D], F32, tag="num_sb")
                nc.vector.tensor_mul(num_sb, q_sb, num_ps)
                nc.gpsimd.dma_start(num_d[ts(i, P), :], num_sb)
                x_sb = work.tile([P, D], F32, tag="x_sb")
                nc.vector.tensor_mul(x_sb, num_sb, rcp_sb)

                # xT via PE transpose, then the output projection
                xT_psum = psum.tile([P, NC, P], F32, tag="xT", bufs=2, name="xT_ps")
                for c in range(NC):
                    nc.tensor.transpose(xT_psum[:, c, :], x_sb[:, ts(c, P)], identity)
                xT_sb = work.tile([P, NC, P], F32R, tag="xT_sb")
                nc.vector.tensor_copy(xT_sb, xT_psum)

                out_ps = proj_psum(xT_sb, "wo", 3 if with_biases else None)
                out_sb = outs.tile([P, D], F32, tag="out_sb")
                nc.vector.tensor_copy(out_sb, out_ps)
                nc.gpsimd.dma_start(out1_d[ts(i, P), :], out_sb)

    nc.finalize()
    return nc


def _pack_band(w_bias, local_mask):
    """[128, NT*384]: pack[t', i*384 + jl*128 + s'] =
    (w_bias*mask)[i*128+s', (i-1+jl)*128+t']  (transposed band blocks)."""
    wbm = np.asarray(w_bias, np.float32) * np.asarray(local_mask, np.float32)
    pack = np.zeros((NT, P, 3 * P), np.float32)
    for i in range(NT):
        for jl, j in _band_blocks(i):
            blk = wbm[i * P : (i + 1) * P, j * P : (j + 1) * P]
            pack[i, :, jl * P : (jl + 1) * P] = blk.T
    # anything |i-j| >= 2 tiles must be zero for the decomposition to hold
    for i in range(NT):
        lo = max(0, (i - 1) * P)
        hi = min(S, (i + 2) * P)
        row = wbm[i * P : (i + 1) * P]
        if row[:, :lo].any() or row[:, hi:].any():
            raise ValueError("w_bias*mask has support outside the 3-tile band")
    return np.ascontiguousarray(pack.transpose(1, 0, 2).reshape(P, NT * 3 * P))


def _pack_seq(x):
    """[S, D] -> [NT, 128, NC*128], pack[i, p, c*128+t] = x[i*128+t, c*128+p]."""
    return np.ascontiguousarray(
        x.reshape(NT, P, NC, P).transpose(0, 3, 2, 1).reshape(NT, P, NC * P)
    )


def _pack_w(w):
    """[D, D] -> [128, NC*512] with pack[p, c*512+n] = w[n, c*128+p]."""
    return np.ascontiguousarray(
        np.asarray(w, np.float32)
        .T.reshape(NC, P, D)
        .transpose(1, 0, 2)
        .reshape(P, NC * D)
    )


_CACHE = {}


def _get_nc(with_biases):
    key = bool(with_biases)
    if key not in _CACHE:
        _CACHE[key] = build(key)
    return _CACHE[key]


def run(inputs, trace=False):
    q = np.asarray(inputs["q"], np.float32)
    k = np.asarray(inputs["k"], np.float32)
    v = np.asarray(inputs["v"], np.float32)
    B = q.shape[0]
    assert B == N_CORES and q.shape[1:] == (S, D)

    biases = np.stack(
        [
            np.asarray(inputs["Wq_b"], np.float32),
            np.asarray(inputs["Wk_b"], np.float32),
            np.asarray(inputs["Wv_b"], np.float32),
            np.asarray(inputs["out_b"], np.float32),
        ]
    )
    with_biases = bool(np.any(biases))

    shared = {
        "wqP": _pack_w(inputs["Wq_w"]),
        "wkP": _pack_w(inputs["Wk_w"]),
        "wvP": _pack_w(inputs["Wv_w"]),
        "woP": _pack_w(inputs["out_w"]),
        "bandP": _pack_band(inputs["w_bias"], inputs["local_mask"]),
    }
    if with_biases:
        shared["biases"] = biases

    in_maps = []
    for b in range(B):
        m = dict(shared)
        m["qP"] = _pack_seq(q[b])
        m["kP"] = _pack_seq(k[b])
        m["vP"] = _pack_seq(v[b])
        in_maps.append(m)

    nc = _get_nc(with_biases)
    res = run_bass_kernel_spmd(
        nc, in_maps, core_ids=list(range(N_CORES)), trace=trace
    )
    out1 = np.stack([res.results[b]["out1"] for b in range(B)])
    num = np.stack([res.results[b]["num"] for b in range(B)])
    return (out1, num), res


def kernel(**inputs):
    (out1, num), _ = run(inputs, trace=False)
    return (out1, num)


# revision 7
# speedup vs baseline: 1.7995x; 1.4230x over previous
"""AFT-local attention on 8 Trainium2 NeuronCores.

Reference (per batch element b, S=2048, D=512, window=128):
    query = q @ Wq.T + bq;  Q_ = sigmoid(query)
    key_p = k @ Wk.T + bk;  ek = exp(key_p)
    value = v @ Wv.T + bv;  ekv = ek * value
    ew    = exp(w_bias * local_mask)          # S x S, == 1 outside the band
    num_raw = ew @ ekv ; den = ew @ ek        # dense S x S einsums
    num  = Q_ * num_raw;  x = num / den
    out1 = x @ out_w.T + out_b
    return (out1, num)

Key decomposition: ew = 1 + (exp(wb_masked) - 1) restricted to the band
|i-j| < 128, so  ew @ Y = colsum(Y) + EWM1_band @ Y  where the banded part
only touches <=3 column tiles of 128 per row tile of 128 (46 block matmuls
instead of 256 dense ones).  colsum(Y) is one ones-vector matmul per
sequence tile, and its broadcast back to all 128 output partitions is a
K=1 matmul accumulated into the same PSUM tile.

Sharding: pure data-parallel; batch B=8 -> one batch element per core.

Matmuls run as float32r (single-pass fp32 on the PE systolic array,
~1.5e-4 rel err measured on silicon).  All DMA sources are host-packed so
every partition line is a single contiguous burst.
"""

import sys

if "/opt/trn_rl_repo" not in sys.path:
    sys.path.insert(0, "/opt/trn_rl_repo")

import numpy as np

import concourse.bacc as bacc
import concourse.mybir as mybir
import concourse.tile as tile
from concourse.bass import ts
from concourse.bass_utils import run_bass_kernel_spmd
from concourse.masks import make_identity

F32 = mybir.dt.float32
F32R = mybir.dt.float32r
EXP = mybir.ActivationFunctionType.Exp
SIGMOID = mybir.ActivationFunctionType.Sigmoid

S = 2048
D = 512
P = 128
NT = S // P  # 16 sequence tiles
NC = D // P  # 4 contraction chunks of the model dim
N_CORES = 8


def _band_blocks(i):
    """Valid (jl, j) column-tile neighbors for row tile i."""
    return [(jl, i - 1 + jl) for jl in range(3) if 0 <= i - 1 + jl < NT]


def build(with_biases):
    nc = bacc.Bacc(None, target_bir_lowering=False, debug=False)

    # host-packed layouts: every [128, ...] DMA partition line is contiguous
    qP_d = nc.dram_tensor("qP", [NT, P, NC * P], F32, kind="ExternalInput")
    kP_d = nc.dram_tensor("kP", [NT, P, NC * P], F32, kind="ExternalInput")
    vP_d = nc.dram_tensor("vP", [NT, P, NC * P], F32, kind="ExternalInput")
    wq_d = nc.dram_tensor("wqP", [P, NC * D], F32, kind="ExternalInput")
    wk_d = nc.dram_tensor("wkP", [P, NC * D], F32, kind="ExternalInput")
    wv_d = nc.dram_tensor("wvP", [P, NC * D], F32, kind="ExternalInput")
    wo_d = nc.dram_tensor("woP", [P, NC * D], F32, kind="ExternalInput")
    band_d = nc.dram_tensor("bandP", [P, NT * 3 * P], F32, kind="ExternalInput")
    if with_biases:
        # rows: bq, bk, bv, bo
        bias_d = nc.dram_tensor("biases", [4, D], F32, kind="ExternalInput")
    out1_d = nc.dram_tensor("out1", [S, D], F32, kind="ExternalOutput")
    num_d = nc.dram_tensor("num", [S, D], F32, kind="ExternalOutput")

    with tile.TileContext(nc) as tc:
        with (
            tc.tile_pool(name="consts", bufs=1) as consts,
            tc.tile_pool(name="weights", bufs=1) as wpool,
            tc.tile_pool(name="ekk_pool", bufs=1) as ekkpool,
            tc.tile_pool(name="kv_in", bufs=3) as kvpool,
            tc.tile_pool(name="q_in", bufs=3) as qpool,
            tc.tile_pool(name="work", bufs=3) as work,
            tc.tile_pool(name="outs", bufs=3) as outs,
            tc.tile_pool(name="psum", bufs=1, space="PSUM") as psum,
        ):
            # ---- weights: wk/wv first so phase B starts ASAP ----
            w_sb = {}

            def load_w(name, d):
                t = wpool.tile([P, NC, D], F32R, tag=f"w_{name}", name=f"w_{name}")
                nc.sync.dma_start(
                    t, d[:, :].bitcast(F32R).rearrange("p (c n) -> p c n", c=NC)
                )
                w_sb[name] = t

            load_w("wk", wk_d)
            load_w("wv", wv_d)

            # ---- constants ----
            identity = consts.tile([P, P], F32)
            make_identity(nc, identity)
            ones_f32 = consts.tile([P, 1], F32)
            nc.gpsimd.memset(ones_f32, 1.0)
            ones_col = consts.tile([P, 1], F32R)  # lhsT for column sums
            nc.vector.tensor_copy(ones_col, ones_f32)
            ones_row = consts.tile([1, P], F32R)  # lhsT for partition bcast
            nc.vector.tensor_copy(ones_row, ones_f32[0:1, 0:1].broadcast_to([1, P]))

            if with_biases:
                bias_sb = consts.tile([4, D], F32R)
                nc.sync.dma_start(bias_sb, bias_d[:, :].bitcast(F32R))

            # ---- ekk: per seq-tile j, [ekv | ek] along free dim ----
            ekk = ekkpool.tile([P, NT, 2 * D], F32R)

            # kv input tiles: emit all DMAs up front so the rings stay fed
            kv_tiles = []
            for j in range(NT):
                kT_t = kvpool.tile([P, NC, P], F32R, tag="kT_t", bufs=5, name="kT_t")
                nc.sync.dma_start(
                    kT_t, kP_d[j].bitcast(F32R).rearrange("p (c t) -> p c t", c=NC)
                )
                vT_t = kvpool.tile([P, NC, P], F32R, tag="vT_t", bufs=5, name="vT_t")
                nc.sync.dma_start(
                    vT_t, vP_d[j].bitcast(F32R).rearrange("p (c t) -> p c t", c=NC)
                )
                kv_tiles.append((kT_t, vT_t))

            load_w("wq", wq_d)
            load_w("wo", wo_d)

            # ---- band -> ewm1, computed in place in two whole-tensor ops ----
            ewm1 = wpool.tile([P, NT, 3 * P], F32R, tag="ewm1", name="ewm1")
            nc.gpsimd.dma_start(
                ewm1, band_d[:, :].bitcast(F32R).rearrange("p (i f) -> p i f", i=NT)
            )
            ew_flat = ewm1.rearrange("p i f -> p (i f)")
            nc.scalar.activation(ew_flat, ew_flat.bitcast(F32), EXP)
            nc.vector.tensor_scalar_add(ew_flat, ew_flat.bitcast(F32), -1.0)

            def proj_psum(xT_tile, wname, bias_row):
                """psum [128, 512] = (x @ W.T + b) for one 128-seq tile."""
                ps = psum.tile([P, D], F32, tag="ps", bufs=3, name="proj_ps")
                for c in range(NC):
                    nc.tensor.matmul(
                        ps,
                        xT_tile[:, c, :],
                        w_sb[wname][:, c, :],
                        start=(c == 0),
                        stop=(c == NC - 1 and bias_row is None),
                    )
                if bias_row is not None:
                    nc.tensor.matmul(
                        ps,
                        ones_row[:, :],
                        bias_sb[bias_row : bias_row + 1, :],
                        start=False,
                        stop=True,
                    )
                return ps

            # ---- phase B: ek / ekv + column sums ----
            csum_ps = {}
            for h in range(2):
                csum_ps[h] = psum.tile([1, D], F32, tag="xT", bufs=2, name=f"cs{h}")
            for j in range(NT):
                kT_t, vT_t = kv_tiles[j]

                keyp_ps = proj_psum(kT_t, "wk", 1 if with_biases else None)
                ek_view = ekk[:, j, D : 2 * D]
                nc.scalar.activation(ek_view, keyp_ps, EXP)

                val_ps = proj_psum(vT_t, "wv", 2 if with_biases else None)
                nc.vector.tensor_mul(ekk[:, j, 0:D], ek_view.bitcast(F32), val_ps)

                # column sums across the whole sequence (shared by all
                # output row tiles): ones^T @ ekk
                for h in range(2):
                    nc.tensor.matmul(
                        csum_ps[h],
                        ones_col[:, :],
                        ekk[:, j, h * D : (h + 1) * D],
                        start=(j == 0),
                        stop=(j == NT - 1),
                    )

            csum_sb = consts.tile([1, 2 * D], F32R)
            for h in range(2):
                nc.vector.tensor_copy(csum_sb[:, h * D : (h + 1) * D], csum_ps[h])

            # ---- phase C: per output row tile, software-pipelined so the
            # PE never waits on the DVE epilogue chain: the transpose +
            # output projection of tile i-1 are emitted after the band
            # matmuls of tile i ----
            def tail_stage(x_sb, i):
                xT_psum = psum.tile([P, NC, P], F32, tag="xT", bufs=2, name="xT_ps")
                for c in range(NC):
                    nc.tensor.transpose(xT_psum[:, c, :], x_sb[:, ts(c, P)], identity)
                xT_sb = work.tile([P, NC, P], F32R, tag="xT_sb")
                nc.vector.tensor_copy(xT_sb, xT_psum)

                out_ps = proj_psum(xT_sb, "wo", 3 if with_biases else None)
                out_sb = outs.tile([P, D], F32, tag="out_sb")
                nc.vector.tensor_copy(out_sb, out_ps)
                nc.gpsimd.dma_start(out1_d[ts(i, P), :], out_sb)

            pending = None
            for i in range(NT):
                qT_t = qpool.tile([P, NC, P], F32R, tag="qT_t", bufs=4, name="qT_t")
                nc.sync.dma_start(
                    qT_t, qP_d[i].bitcast(F32R).rearrange("p (c t) -> p c t", c=NC)
                )
                query_ps = proj_psum(qT_t, "wq", 0 if with_biases else None)
                q_sb = work.tile([P, D], F32, tag="q_sb")
                nc.scalar.activation(q_sb, query_ps, SIGMOID)

                halves = []
                for h in range(2):  # 0: num (ekv), 1: den (ek)
                    ps = psum.tile([P, D], F32, tag="bps", bufs=3, name=f"bps{h}")
                    blocks = _band_blocks(i)
                    for bi, (jl, j) in enumerate(blocks):
                        nc.tensor.matmul(
                            ps,
                            ewm1[:, i, ts(jl, P)],
                            ekk[:, j, h * D : (h + 1) * D],
                            start=(bi == 0),
                            stop=False,
                        )
                    # + colsum broadcast to all 128 partitions (K=1 matmul)
                    nc.tensor.matmul(
                        ps,
                        ones_row[:, :],
                        csum_sb[:, h * D : (h + 1) * D],
                        start=False,
                        stop=True,
                    )
                    halves.append(ps)
                num_ps, den_ps = halves

                rcp_sb = work.tile([P, D], F32, tag="rcp")
                nc.vector.reciprocal_approx_fast(out=rcp_sb, in_=den_ps)
                num_sb = outs.tile([P, D], F32, tag="num_sb")
                nc.vector.tensor_mul(num_sb, q_sb, num_ps)
                nc.gpsimd.dma_start(num_d[ts(i, P), :], num_sb)
                x_sb = work.tile([P, D], F32, tag="x_sb")
                nc.vector.tensor_mul(x_sb, num_sb, rcp_sb)

                if pending is not None:
                    tail_stage(*pending)
                pending = (x_sb, i)
            tail_stage(*pending)

    nc.finalize()
    return nc


def _pack_band(w_bias, local_mask):
    """[128, NT*384]: pack[t', i*384 + jl*128 + s'] =
    (w_bias*mask)[i*128+s', (i-1+jl)*128+t']  (transposed band blocks)."""
    wbm = np.asarray(w_bias, np.float32) * np.asarray(local_mask, np.float32)
    pack = np.zeros((NT, P, 3 * P), np.float32)
    for i in range(NT):
        for jl, j in _band_blocks(i):
            blk = wbm[i * P : (i + 1) * P, j * P : (j + 1) * P]
            pack[i, :, jl * P : (jl + 1) * P] = blk.T
    # anything |i-j| >= 2 tiles must be zero for the decomposition to hold
    for i in range(NT):
        lo = max(0, (i - 1) * P)
        hi = min(S, (i + 2) * P)
        row = wbm[i * P : (i + 1) * P]
        if row[:, :lo].any() or row[:, hi:].any():
            raise ValueError("w_bias*mask has support outside the 3-tile band")
    return np.ascontiguousarray(pack.transpose(1, 0, 2).reshape(P, NT * 3 * P))


def _pack_seq(x):
    """[S, D] -> [NT, 128, NC*128], pack[i, p, c*128+t] = x[i*128+t, c*128+p]."""
    return np.ascontiguousarray(
        x.reshape(NT, P, NC, P).transpose(0, 3, 2, 1).reshape(NT, P, NC * P)
    )


def _pack_w(w):
    """[D, D] -> [128, NC*512] with pack[p, c*512+n] = w[n, c*128+p]."""
    return np.ascontiguousarray(
        np.asarray(w, np.float32)
        .T.reshape(NC, P, D)
        .transpose(1, 0, 2)
        .reshape(P, NC * D)
    )


_CACHE = {}


def _get_nc(with_biases):
    key = bool(with_biases)
    if key not in _CACHE:
        _CACHE[key] = build(key)
    return _CACHE[key]


def run(inputs, trace=False):
    q = np.asarray(inputs["q"], np.float32)
    k = np.asarray(inputs["k"], np.float32)
    v = np.asarray(inputs["v"], np.float32)
    B = q.shape[0]
    assert B == N_CORES and q.shape[1:] == (S, D)

    biases = np.stack(
        [
            np.asarray(inputs["Wq_b"], np.float32),
            np.asarray(inputs["Wk_b"], np.float32),
            np.asarray(inputs["Wv_b"], np.float32),
            np.asarray(inputs["out_b"], np.float32),
        ]
    )
    with_biases = bool(np.any(biases))

    shared = {
        "wqP": _pack_w(inputs["Wq_w"]),
        "wkP": _pack_w(inputs["Wk_w"]),
        "wvP": _pack_w(inputs["Wv_w"]),
        "woP": _pack_w(inputs["out_w"]),
        "bandP": _pack_band(inputs["w_bias"], inputs["local_mask"]),
    }
    if with_biases:
        shared["biases"] = biases

    in_maps = []
    for b in range(B):
        m = dict(shared)
        m["qP"] = _pack_seq(q[b])
        m["kP"] = _pack_seq(k[b])
        m["vP"] = _pack_seq(v[b])
        in_maps.append(m)

    nc = _get_nc(with_biases)
    res = run_bass_kernel_spmd(
        nc, in_maps, core_ids=list(range(N_CORES)), trace=trace
    )
    out1 = np.stack([res.results[b]["out1"] for b in range(B)])
    num = np.stack([res.results[b]["num"] for b in range(B)])
    return (out1, num), res


def kernel(**inputs):
    (out1, num), _ = run(inputs, trace=False)
    return (out1, num)
